# revision 24
# baseline (speedup 1.0000x reference)
"""GQA kernel for Trainium2, 8 NeuronCores, tensor-parallel over heads.

Problem: B=1, T=2048, C=4096, 32 q-heads, 16 kv-heads, head_dim=128,
scale = 1/sqrt(32), causal. q head H uses kv head H%16.

Sharding (no collectives needed): core c owns q-heads
{2c, 2c+1, 2c+16, 2c+17} and kv-heads {2c, 2c+1}. Each output column
block depends only on its own head, so the full output is a host-side
concat of per-core column slices.

Per-core schedule (all matmuls bf16, fp32 PSUM accumulation):
  Load phase: x streamed per-kc-chunk (32 x 512KB); q0+k0 strips (8
  open PSUM accumulations) consume chunks kc-major at DMA rate.
  Post-load: remaining projection strips form a "filler" stream that
  is interleaved into the attention blocks so the PE never waits for
  the Scalar engine's exp (1.1us per [128,1024] tile).
  Attention per (head, Tq-block b): iters p: [drain filler] QK(p)
  pair -> exp(p) on ACT -> PV(p-1) accumulation; diagonal QK streams
  shrunk to the unmasked column range; per-tile triangular mask via
  DVE multiply only on the s==r diagonal 128x128 tile.
  V tiles transposed via XBAR DMA transpose (no PE/DVE cost).
  Row sums via ones-column appended to v (VROW=129), normalized with
  DVE reciprocal+scalar-mul, DMA'd out fp32.
"""

import numpy as np
import ml_dtypes

BF16 = ml_dtypes.bfloat16
T = 2048
C = 4096
D = 128
N_HEADS = 32
N_KV = 16
SCALE = float(1.0 / np.sqrt(np.float32(N_HEADS)))
KC = C // 128          # 32 contraction chunks
NQH = 4                # local q heads per core
NKV = 2                # local kv heads per core
NT = T // 128          # 16 token tiles
VROW = D + 1           # 129: v with ones column
VSTRIDE = 144          # vt per-tile column stride (288B, 32B-aligned for XBAR)
N_CORES = 8

_prog_cache = {}


def _build_program():
    if "nc" in _prog_cache:
        return _prog_cache["nc"]
    import concourse.bass as bass
    import concourse.tile as tile
    from concourse import bacc, mybir

    dt = mybir.dt
    f32 = dt.float32
    bf16 = dt.bfloat16
    EXP = mybir.ActivationFunctionType.Exp

    nc = bacc.Bacc("TRN2", target_bir_lowering=False, debug=False,
                   num_devices=N_CORES)

    xT_d = nc.dram_tensor("xT", [128, KC * T], bf16, kind="ExternalInput").ap()
    wq_d = nc.dram_tensor("wq", [NQH, 128, C], bf16, kind="ExternalInput").ap()
    wk_d = nc.dram_tensor("wk", [NKV, 128, C], bf16, kind="ExternalInput").ap()
    wv_d = nc.dram_tensor("wv", [NKV, 128, C], bf16, kind="ExternalInput").ap()
    mask_d = nc.dram_tensor("masks", [128, 128], bf16,
                            kind="ExternalInput").ap()
    out_d = nc.dram_tensor("out", [T, NQH * D], bf16,
                       kind="ExternalOutput").ap()

    with tile.TileContext(nc) as tc:
        with (
            tc.tile_pool(name="persist", bufs=1) as persist,
            tc.tile_pool(name="wpool", bufs=4) as wpool,
            tc.tile_pool(name="ptpool", bufs=3) as ptpool,
            tc.tile_pool(name="opool", bufs=8) as opool,
            tc.tile_pool(name="recpool", bufs=4) as recpool,
            tc.tile_pool(name="pvspool", bufs=4,
                         space=bass.MemorySpace.PSUM) as pvspool,
            tc.tile_pool(name="fillpool", bufs=2,
                         space=bass.MemorySpace.PSUM) as fillpool,
            tc.tile_pool(name="spppool", bufs=1,
                         space=bass.MemorySpace.PSUM) as spppool,
        ):
            tri = persist.tile([128, 128], bf16, name="tri", tag="tri")
            qt = persist.tile([128, NQH * T], bf16, name="qt", tag="qt")
            kt = persist.tile([128, NKV * T], bf16, name="kt", tag="kt")
            vt = persist.tile([128, NKV * NT * VSTRIDE], bf16, name="vt",
                              tag="vt")
            vts = persist.tile([128, T], bf16, name="vts", tag="vts")
            xks = [persist.tile([128, T], bf16, name=f"xk{i}", tag=f"xk{i}")
                   for i in range(KC)]

            # ones columns of v (row-sum trick)
            for i in range(NKV * NT):
                nc.vector.memset(vt[:, i * VSTRIDE + D: i * VSTRIDE + VROW],
                                 1.0)

            # ---- DMA helpers ----
            wts = {}

            def dma_w(src, idx, key, chunks=1):
                w = wpool.tile([128, C], bf16, name=f"w_{key}", tag="w")
                csz = C // chunks
                for ci in range(chunks):
                    nc.sync.dma_start(
                        out=w[:, ci * csz:(ci + 1) * csz],
                        in_=src[idx][:, ci * csz:(ci + 1) * csz])
                wts[key] = w

            def dma_x(kc, halves=False, eng=None):
                eng = eng or nc.sync
                if halves:
                    for hh in range(2):
                        eng.dma_start(
                            out=xks[kc][:, hh * 1024:(hh + 1) * 1024],
                            in_=xT_d[:, kc * T + hh * 1024:
                                     kc * T + (hh + 1) * 1024])
                else:
                    eng.dma_start(out=xks[kc][:],
                                  in_=xT_d[:, kc * T:(kc + 1) * T])

            # DMA issue order: first-needed data first, chunked fine.
            # wpool slot rotation (bufs=3): q0,k0,v0 -> q2,q1,k1 -> v1,q3.
            # Late-weight DMAs are issued only after the slot's previous
            # owner's last reader is issued (WAR deps are issue-order
            # based), i.e. after the load loop / as filler items.
            wq0 = wpool.tile([128, C], bf16, name="w_q0", tag="w")
            wk0 = wpool.tile([128, C], bf16, name="w_k0", tag="w")
            wts["q0"], wts["k0"] = wq0, wk0

            def wchunk(w, src, ci):
                nc.sync.dma_start(out=w[:, ci * 1024:(ci + 1) * 1024],
                                  in_=src[:, ci * 1024:(ci + 1) * 1024])

            # weight chunk ci is first needed at kc=8*ci; interleave the
            # chunks between x tiles so x arrival stays ahead of the PE.
            wchunk(wq0, wq_d[0], 0)
            dma_x(0, halves=True)
            dma_x(1, eng=nc.scalar)
            wchunk(wk0, wk_d[0], 0)
            dma_x(2)
            dma_x(3, eng=nc.scalar)
            wchunk(wq0, wq_d[0], 1)
            dma_x(4)
            dma_x(5, eng=nc.scalar)
            wchunk(wk0, wk_d[0], 1)
            wchunk(wq0, wq_d[0], 2)
            dma_x(6)
            dma_x(7, eng=nc.scalar)
            wchunk(wk0, wk_d[0], 2)
            dma_x(8)
            dma_x(9, eng=nc.scalar)
            wchunk(wq0, wq_d[0], 3)
            dma_x(10)
            dma_x(11, eng=nc.scalar)
            wchunk(wk0, wk_d[0], 3)
            for kc in range(12, 16):
                dma_x(kc)
            dma_w(wv_d, 0, "v0")
            nc.sync.dma_start(out=tri[:], in_=mask_d[:])
            for kc in range(16, KC):
                dma_x(kc)
            dma_w(wq_d, 2, "q2")

            # ---- load phase: q0 + k0 strips consume x chunks kc-major ----
            q0w, k0w = wts["q0"], wts["k0"]
            q0ps = [pvspool.tile([128, 512], f32, name=f"ld_q0_{t4}",
                                 tag="pvs") for t4 in range(4)]
            k0ps01 = [fillpool.tile([128, 512], f32, name=f"ld_k0_{t4}",
                                    tag="fill") for t4 in range(2)]
            k0pair = spppool.tile([128, 1024], f32, name="ld_k0_23",
                                  tag="spp")
            strips = (
                [(q0w, q0ps[t4][:], t4) for t4 in range(4)]
                + [(k0w, k0ps01[t4][:], t4) for t4 in range(2)]
                + [(k0w, k0pair[:, (t4 - 2) * 512:(t4 - 1) * 512], t4)
                   for t4 in (2, 3)]
            )
            with nc.named_scope("load"):
                for kc in range(KC):
                    for w, ps, t4 in strips:
                        nc.tensor.matmul(
                            ps,
                            lhsT=w[:, kc * 128:(kc + 1) * 128],
                            rhs=xks[kc][:, t4 * 512:(t4 + 1) * 512],
                            start=(kc == 0), stop=(kc == KC - 1),
                        )
                # closes: fill slots first (v0's strips reuse them at
                # once), then spp (attn(0,0) QK), then pvs (attn PV)
                for t4 in range(2):
                    nc.vector.tensor_copy(
                        out=kt[:, t4 * 512:(t4 + 1) * 512],
                        in_=k0ps01[t4][:])
                for t4 in (2, 3):
                    nc.vector.tensor_copy(
                        out=kt[:, t4 * 512:(t4 + 1) * 512],
                        in_=k0pair[:, (t4 - 2) * 512:(t4 - 1) * 512])
                for t4 in range(4):
                    nc.vector.tensor_copy(
                        out=qt[:, t4 * 512:(t4 + 1) * 512],
                        in_=q0ps[t4][:])
            del wts["q0"], wts["k0"]
            # q0/k0 slots are fully consumed at load end: safe to queue
            # their replacement weight DMAs now.
            dma_w(wq_d, 1, "q1")
            dma_w(wk_d, 1, "k1")

            # ---- filler stream (post-load projection strips) ----
            # item kinds: ('mm', fn) counted by drain(); ('free', fn) not.
            filler = []
            markers = {}
            pending = []  # deferred (items, marker_name) from v strips

            def flush_items():
                for items, _ in pending:
                    filler.extend(items)

            def flush_markers():
                for _, mname in pending:
                    markers[mname] = len(filler)
                del pending[:]

            def flush_pending():
                flush_items()
                flush_markers()

            def strip_items(key, t4, dest, dbase, vkey=None):
                """Append one [128,512] strip (32 MMs + close [+ 4 DMA
                transposes for v strips, deferred into the next strip]) to
                the filler stream."""
                state = {}

                def first_mm(kc=0):
                    w = wts[key]
                    ps = fillpool.tile([128, 512], f32,
                                       name=f"ps_{key}_{t4}", tag="fill")
                    state["ps"] = ps
                    state["w"] = w
                    nc.tensor.matmul(
                        ps[:], lhsT=w[:, 0:128],
                        rhs=xks[0][:, t4 * 512:(t4 + 1) * 512],
                        start=True, stop=False)

                def mk_mm(kc):
                    def mm():
                        nc.tensor.matmul(
                            state["ps"][:],
                            lhsT=state["w"][:, kc * 128:(kc + 1) * 128],
                            rhs=xks[kc][:, t4 * 512:(t4 + 1) * 512],
                            start=False, stop=(kc == KC - 1))
                    return mm

                def close():
                    nc.vector.tensor_copy(
                        out=dest[:, dbase + t4 * 512:dbase + (t4 + 1) * 512],
                        in_=state["ps"][:])

                filler.append(("mm", first_mm))
                for kc in range(1, 8):
                    filler.append(("mm", mk_mm(kc)))
                flush_items()
                for kc in range(8, 12):
                    filler.append(("mm", mk_mm(kc)))
                flush_markers()
                for kc in range(12, KC):
                    filler.append(("mm", mk_mm(kc)))
                filler.append(("free", close))
                if vkey is not None:
                    kv = vkey
                    items = []
                    for j in range(4 * t4, 4 * t4 + 4):
                        def mk_dtr(j=j):
                            def dtr():
                                nc.sync.dma_start_transpose(
                                    out=vt[:, (kv * NT + j) * VSTRIDE:
                                           (kv * NT + j) * VSTRIDE + D],
                                    in_=vts[:, j * 128:(j + 1) * 128])
                            return dtr
                        items.append(("free", mk_dtr(j)))
                    pending.append((items, f"v{vkey}t{t4}"))
                else:
                    markers[f"{key}t{t4}"] = len(filler)

            # order matters: each strip must be appended before the
            # attention blocks that depend on it (see block order below),
            # and each late-weight DMA after its wpool slot's previous
            # owner's last strip.
            for t4 in range(4):
                strip_items("v0", t4, vts, 0, vkey=0)
            filler.append(("free", lambda: dma_w(wv_d, 1, "v1")))
            for t4 in range(4):
                strip_items("q2", t4, qt, 2 * T)
            filler.append(("free", lambda: dma_w(wq_d, 3, "q3")))
            for t4 in (1, 2, 3):
                strip_items("q1", t4, qt, T)
            for t4 in range(4):
                strip_items("k1", t4, kt, T)
            for t4 in range(4):
                strip_items("v1", t4, vts, 0, vkey=1)
            for t4 in (1, 2, 3):
                strip_items("q3", t4, qt, 3 * T)
            strip_items("q1", 0, qt, T)
            strip_items("q3", 0, qt, 3 * T)
            flush_pending()

            fpos = [0]

            def drain(n_mm):
                """Issue filler items until n_mm 'mm' items issued (or
                filler exhausted). 'free' items issue without counting."""
                while n_mm > 0 and fpos[0] < len(filler):
                    kind, fn = filler[fpos[0]]
                    fn()
                    fpos[0] += 1
                    if kind == "mm":
                        n_mm -= 1

            def drain_to(marker):
                while fpos[0] < markers[marker]:
                    kind, fn = filler[fpos[0]]
                    fn()
                    fpos[0] += 1

            # ---- attention ----
            pending_out = []

            def flush_out():
                for ot, oh, ob, s in pending_out:
                    nc.sync.dma_start(
                        out=out_d[ob * 512 + s * 128:
                                  ob * 512 + (s + 1) * 128,
                                  oh * D:(oh + 1) * D],
                        in_=ot[:])
                del pending_out[:]

            def attn_block(h, b, immediate_out=False):
                kv = h % 2
                with nc.named_scope(f"attn_{h}_{b}"):
                    flush_out()
                    pvs = [pvspool.tile([128, 512], f32,
                                        name=f"pv_{h}_{b}_{s}", tag="pvs")
                           for s in range(4)]
                    P = 2 * b + 2
                    pts = {}

                    def emit_qk(p):
                        spp = spppool.tile([128, 1024], f32,
                                           name=f"sp_{h}_{b}_{p}", tag="spp")
                        for half in range(2):
                            j = 2 * p + half
                            r = j - 4 * b
                            lo = max(r, 0) * 128
                            nc.tensor.matmul(
                                spp[:, half * 512 + lo:(half + 1) * 512],
                                lhsT=kt[:, kv * T + j * 128:
                                        kv * T + (j + 1) * 128],
                                rhs=qt[:, h * T + b * 512 + lo:
                                       h * T + (b + 1) * 512],
                                start=True, stop=True,
                            )
                        pt = ptpool.tile([128, 1024], bf16,
                                         name=f"pt_{h}_{b}_{p}", tag="pt")
                        nc.scalar.activation(pt[:], spp[:], EXP, scale=SCALE)
                        for half in range(2):
                            j = 2 * p + half
                            r = j - 4 * b
                            if 0 <= r <= 3:
                                sl = pt[:, half * 512 + r * 128:
                                        half * 512 + (r + 1) * 128]
                                nc.vector.tensor_mul(sl, sl, tri[:])
                        pts[p] = pt

                    def emit_pv(p):
                        pt = pts.pop(p)
                        for half in range(2):
                            j = 2 * p + half
                            r = j - 4 * b
                            vsl = vt[:, (kv * NT + j) * VSTRIDE:
                                     (kv * NT + j) * VSTRIDE + VROW]
                            for s in range(max(0, r), 4):
                                nc.tensor.matmul(
                                    pvs[s][:, 0:VROW],
                                    lhsT=pt[:, half * 512 + s * 128:
                                            half * 512 + (s + 1) * 128],
                                    rhs=vsl,
                                    start=(j == 0), stop=(j == 4 * b + s),
                                )

                    for p in range(P):
                        if p == 0:
                            drain(3)
                        elif p == 1:
                            drain(7)
                        else:
                            drain(5)
                        emit_qk(p)
                        if p >= 1:
                            emit_pv(p - 1)
                    emit_pv(P - 1)
                    for s in range(4):
                        rec = recpool.tile([128, 1], f32,
                                           name=f"rec_{h}_{b}_{s}", tag="rec")
                        nc.vector.reciprocal(rec[:], pvs[s][:, D:D + 1])
                        ot = opool.tile([128, 128], bf16,
                                        name=f"ot_{h}_{b}_{s}", tag="ot")
                        nc.vector.tensor_scalar_mul(ot[:], pvs[s][:, 0:D],
                                                    rec[:])
                        if immediate_out:
                            nc.sync.dma_start(
                                out=out_d[b * 512 + s * 128:
                                          b * 512 + (s + 1) * 128,
                                          h * D:(h + 1) * D],
                                in_=ot[:])
                        else:
                            pending_out.append((ot, h, b, s))

            # block order with filler-dependency barriers
            block_reqs = [
                ((0, 0), "v0t0"), ((0, 1), "v0t1"),
                ((0, 2), "v0t2"), ((0, 3), "v0t3"),
                ((2, 0), "q2t0"), ((2, 1), "q2t1"),
                ((2, 2), "q2t2"), ((2, 3), "q2t3"),
                ((1, 1), "v1t1"), ((1, 2), "v1t2"),
                ((1, 3), "v1t3"),
                ((3, 1), "q3t1"), ((3, 2), "q3t2"),
                ((3, 3), "q3t3"),
                ((1, 0), "q1t0"), ((3, 0), "q3t0"),
            ]
            for i, ((h, b), req) in enumerate(block_reqs):
                drain_to(req)
                attn_block(h, b, immediate_out=(i >= len(block_reqs) - 2))
            flush_out()
            drain(10 ** 9)  # flush any leftover filler

    nc.compile()
    _prog_cache["nc"] = nc
    return nc


def _host_prep(x, Wq, bq, Wk, bk, Wv, bv):
    """Shard + repack inputs for the 8 cores. Returns in_maps list."""
    assert x.shape == (1, T, C)
    assert np.abs(bq).max() == 0 and np.abs(bk).max() == 0, \
        "nonzero q/k biases not supported"

    x0 = np.ascontiguousarray(x[0]).astype(BF16)
    # xT packed: [128, kc*T + t] = x[t, 128*kc + p]
    xT = np.ascontiguousarray(
        x0.reshape(T, KC, 128).transpose(2, 1, 0).reshape(128, KC * T))

    # per-tile triangular causal mask: tri[tk, tq] = tq >= tk
    tq = np.arange(128)[None, :]
    tk = np.arange(128)[:, None]
    tri = np.ascontiguousarray((tq >= tk).astype(BF16))

    def pack_w(Wrows):
        # Wrows: [128 (out c), C (in)] for one head ->
        # packed[p, 128*kc + c] = Wrows[c, 128*kc + p]
        return np.ascontiguousarray(
            Wrows.astype(BF16).reshape(128, KC, 128).transpose(2, 1, 0)
            .reshape(128, C))

    in_maps = []
    for c in range(N_CORES):
        qheads = [2 * c, 2 * c + 1, 2 * c + 16, 2 * c + 17]
        kvheads = [2 * c, 2 * c + 1]
        wq = np.stack([pack_w(Wq[128 * H:128 * (H + 1)]) for H in qheads])
        wk = np.stack([pack_w(Wk[128 * K:128 * (K + 1)]) for K in kvheads])
        wv = np.stack([pack_w(Wv[128 * K:128 * (K + 1)]) for K in kvheads])
        in_maps.append({
            "xT": xT, "wq": wq, "wk": wk, "wv": wv, "masks": tri,
        })
    return in_maps


def _assemble(results, bv):
    out = np.empty((T, C), dtype=np.float32)
    for c in range(N_CORES):
        r = np.asarray(results[c]["out"]).astype(np.float32)
        qheads = [2 * c, 2 * c + 1, 2 * c + 16, 2 * c + 17]
        for i, H in enumerate(qheads):
            blk = r[:, 128 * i:128 * (i + 1)]
            if bv is not None:
                blk = blk + bv[128 * (H % N_KV_IDX):128 * (H % N_KV_IDX) + 128]
            out[:, 128 * H:128 * (H + 1)] = blk
    return out.reshape(1, T, C)


N_KV_IDX = 16


def _install_trace_hooks():
    """The agent image's antenv lacks axon_hooks; recreate it so
    run_bass_kernel_spmd's trace=True path can capture NTFF profiles."""
    import sys
    import types
    import antenv
    if "antenv.axon_hooks" not in sys.modules:
        mod = types.ModuleType("antenv.axon_hooks")
        mod._hook = None

        def set_axon_ntff_profile_hook(h):
            mod._hook = h

        def get_axon_ntff_profile_hook():
            return mod._hook

        mod.set_axon_ntff_profile_hook = set_axon_ntff_profile_hook
        mod.get_axon_ntff_profile_hook = get_axon_ntff_profile_hook
        sys.modules["antenv.axon_hooks"] = mod
        antenv.axon_hooks = mod
    from antenv.axon_hooks import (get_axon_ntff_profile_hook,
                                   set_axon_ntff_profile_hook)
    if get_axon_ntff_profile_hook() is None:
        if "/root/.axon_site" not in sys.path:
            sys.path.insert(0, "/root/.axon_site")
        from trn_agent_boot.trn_boot import _ntff_profile_via_ctypes
        set_axon_ntff_profile_hook(
            _ntff_profile_via_ctypes("/opt/axon/libaxon_pjrt.so"))
    import concourse.bass_utils as bu
    bu.upload_artifacts = lambda tmpdir: tmpdir


def _run(inputs, trace=False, trace_kwargs=None):
    if trace:
        _install_trace_hooks()
    from concourse.bass_utils import run_bass_kernel_spmd
    nc = _build_program()
    in_maps = _host_prep(**inputs)
    res = run_bass_kernel_spmd(
        nc, in_maps, list(range(N_CORES)), trace=trace,
        **(trace_kwargs or {}))
    bv = inputs["bv"].astype(np.float32)
    bv = bv if np.abs(bv).max() > 0 else None
    out = _assemble(res.results, bv)
    return out, res


def kernel(x, Wq, bq, Wk, bk, Wv, bv):
    out, _ = _run(dict(x=np.asarray(x), Wq=np.asarray(Wq), bq=np.asarray(bq),
                       Wk=np.asarray(Wk), bk=np.asarray(bk),
                       Wv=np.asarray(Wv), bv=np.asarray(bv)))
    return out
